# revision 32
# baseline (speedup 1.0000x reference)
"""DeepseekV3 attention (B=1, S=2048, D=2048, H=16, KV=4) on 8 trn2 cores.

Sharding: token-modulo-8 split. Core c owns query tokens {t : t % 8 == c}
(256 each) — causal attention work is identical on every core, so one SPMD
program serves all 8 cores with per-core DATA (host-sliced hidden columns,
cos/sin slices, causal band masks) carrying the differences.

Per core:
  - kv path replicated: ckv^T = wkv_a^T @ h^T over all 2048 tokens, RMS,
    k_nope^T / v via wkv_b, RoPE on k_rot.
  - q path token-split: q_a^T/q^T only for the core's 256 tokens.
  - attention: scores computed transposed [k, q] (lhsT = k^T tiles), softmax
    denominator via ones-matmul, AV with v in natural [token, dv] layout.
    Causal masking: additive band masks (input data) on the diagonal bands.
  - o_proj over the core's 256 token rows; host reassembles rows.

All matmuls run in bf16 (1 cycle/row on the PE at any free size, vs fp32r's
2-4 and heavier power throttling) with fp32 PSUM accumulation; weights and
hidden states are cast to bf16 host-side, which also halves HBM traffic.
"""
import math
import sys
import types

import ml_dtypes
import numpy as np

# ---------------------------------------------------------------------------
# Container compat: this walrus build rejects instructions carrying more than
# one sync-wait command. Patch Tile to (a) split multi-wait instructions into
# single-wait NoOps on the same engine, (b) hoist the end-of-kernel drain's
# waits onto single-wait NOPs. Also register the NTFF profile hook (the
# image's antenv lacks axon_hooks) so trace=True works for profiling.
# ---------------------------------------------------------------------------
import concourse.bass as bass
import concourse.mybir as mybir
import concourse.tile as tile
from concourse.bass_utils import run_bass_kernel_spmd
from concourse.tile import ScopedClock
from bass_rust import VectorClock

N_PROCS = len(VectorClock())
_PATCHED = False


def _install_ntff_hook():
    if 'antenv.axon_hooks' in sys.modules:
        return
    m = types.ModuleType('antenv.axon_hooks')
    holder = [None]
    m.set_axon_ntff_profile_hook = lambda h: holder.__setitem__(0, h)
    m.get_axon_ntff_profile_hook = lambda: holder[0]
    sys.modules['antenv.axon_hooks'] = m
    try:
        from trn_agent_boot.trn_boot import _ntff_profile_via_ctypes
        m.set_axon_ntff_profile_hook(
            _ntff_profile_via_ctypes('/opt/axon/libaxon_pjrt.so'))
    except Exception:
        pass


def _patched_drain_and_barrier(self, tick_clock, wait_clock):
    gc = tick_clock.global_clock
    for p in range(N_PROCS):
        if gc[p] == 0:
            continue
        single = VectorClock([gc[q] if q == p else 0 for q in range(N_PROCS)])
        nop_inst = self.nc.sync.nop(nofuse=True)
        wait_clock.add_sem_waits(nop_inst.ins, ScopedClock({None: single}))
    self.nc.sync.drain()
    self.nc.all_engine_barrier()
    popped = self.nc._tile_sem_poison_stack.pop()
    assert popped is self._sem_poison
    self.nc.clear_and_free_semaphores(list(self.sems.allocated().values()))
    self.nc.all_engine_barrier()


def _make_split_lower(orig):
    def _split_multi_waits(self, ordered):
        nc = self.nc
        for bb_name, insts in ordered.items():
            out = []
            for inst in insts:
                si = inst.sync_info
                waits = list(si.on_wait) if si is not None else []
                if len(waits) > 1:
                    for w in waits[:-1]:
                        nop = mybir.InstNoOp(
                            name=f"{inst.name}-waitsplit-{nc.next_id()}",
                            engine=inst.engine,
                            sync_info=mybir.SyncInfo(on_wait=[w], on_update=[]),
                        )
                        nc.register_instruction(nop)
                        out.append(nop)
                    inst.sync_info = mybir.SyncInfo(
                        on_wait=[waits[-1]], on_update=list(si.on_update))
                out.append(inst)
            ordered[bb_name] = out
        return orig(self, ordered)
    return _split_multi_waits


def _install_patches():
    global _PATCHED
    _install_ntff_hook()
    if _PATCHED:
        return
    tile.TileContext._drain_and_barrier = _patched_drain_and_barrier
    tile.TileContext._lower_ordered_insts = _make_split_lower(
        tile.TileContext._lower_ordered_insts)
    _PATCHED = True


_install_patches()

# ---------------------------------------------------------------------------
# Problem constants (hardcoded per the spec).
# ---------------------------------------------------------------------------
S = 2048
D = 2048
H = 16
KV = 4
GROUPS = H // KV
DN = 128          # d_nope
DR = 64           # d_rope
DQK = DN + DR     # 192
DV = 128
QR = 1536         # q rank
KVR = 512         # kv rank
EPS = 1e-6
NC_ = 8           # cores
TPC = S // NC_    # 256 tokens per core
NB = S // 128     # 16 k-subtiles
SCALE = 1.0 / math.sqrt(DQK)
NEG = -1e30

F32 = mybir.dt.float32
F32R = mybir.dt.float32r
BF16 = mybir.dt.bfloat16

_BUILT = None     # cached (nc,) so repeat kernel() calls skip rebuild
LAST_RESULTS = None  # BassKernelResults stash for test.py


def _build():
    nc = bass.Bass()

    # ---- DRAM I/O (identical declaration on all cores; data differs) ----
    hT = nc.dram_tensor("hT", [D, S], BF16, kind="ExternalInput")
    hTq = nc.dram_tensor("hTq", [D, TPC], BF16, kind="ExternalInput")
    wqa = nc.dram_tensor("wqa", [D, QR], BF16, kind="ExternalInput")
    wqbm = nc.dram_tensor("wqbm", [QR, H * DQK], BF16, kind="ExternalInput")
    wkva = nc.dram_tensor("wkva", [D, KVR + DR], BF16, kind="ExternalInput")
    wkvbk = nc.dram_tensor("wkvbk", [KVR, KV * DN], BF16, kind="ExternalInput")
    wkvbv = nc.dram_tensor("wkvbv", [KVR, KV * DV], BF16, kind="ExternalInput")
    wo_t = nc.dram_tensor("wo", [H * DV, D], BF16, kind="ExternalInput")
    cossinT = nc.dram_tensor("cossinT", [2 * DR, S], F32, kind="ExternalInput")
    cosq2 = nc.dram_tensor("cosq2", [2 * DR, TPC], BF16, kind="ExternalInput")
    sinq2 = nc.dram_tensor("sinq2", [2 * DR, TPC], BF16, kind="ExternalInput")
    bmask = nc.dram_tensor("bmask", [128, 16 * 64], F32, kind="ExternalInput")
    out = nc.dram_tensor("out", [TPC, D], F32, kind="ExternalOutput")
    # scratch for free->partition broadcasts
    scr_k = nc.dram_tensor("scr_k", [1, S], F32, kind="Internal")
    scr_q = nc.dram_tensor("scr_q", [1, TPC], F32, kind="Internal")
    scr_r = nc.dram_tensor("scr_r", [8, 2 * TPC], F32, kind="Internal")

    def bcast_src(dram, off, ncols):
        # element off.. of the flat DRAM vector, broadcast to 128 partitions
        ap = dram[:]
        return bass.AP(tensor=ap.tensor, offset=ap.offset + off,
                       ap=[[0, 128], [1, ncols]])

    def strided(ap_src, start, stride, count):
        # free-dim strided view of a full 2D sbuf/psum tile AP
        return bass.AP(tensor=ap_src.tensor, offset=ap_src.offset + start,
                       ap=[ap_src.ap[0], [stride, count]])

    with tile.TileContext(nc) as tc:
        with (
            tc.tile_pool(name="persist", bufs=1) as P,   # attention-lived
            tc.tile_pool(name="wstream", bufs=3) as WS,  # streamed weights
            tc.tile_pool(name="ppool", bufs=6) as PP,    # p tiles (bf16)
        ):
            ones_b = P.tile([128, 1], BF16, name="ones_b")
            nc.vector.memset(ones_b[:], 1.0)
            eps_sb = P.tile([1, 1], F32, name="eps_sb")
            nc.vector.memset(eps_sb[:], EPS)
            # bmask[k, kb, i]: causal band masks over the 64 interleaved
            # packed columns at the diagonal of key block kb
            bmask_sb = P.tile([128, 16, 64], F32, name="bmask_sb")
            nc.sync.dma_start(bmask_sb[:], bmask[:, :].rearrange(
                "k (r i) -> k r i", r=16))

            # attention-lived products; qnT2[j] holds the pair (2j, 2j+1)
            # nope queries interleaved: col 2q+p <-> (query q, head 2j+p)
            qnT2 = [P.tile([128, 2 * TPC], BF16, name=f"qnT2{j}")
                    for j in range(8)]
            # qr2[j]: roped queries, zero-padded interleave — rows 0:64 hold
            # head 2j at even cols, rows 64:128 head 2j+1 at odd cols, so one
            # matmul against the duplicated krot2 yields both heads' rope
            # scores at the packed columns.
            qr2 = [P.tile([128, 2 * TPC], BF16, name=f"qr2{j}")
                   for j in range(8)]
            for j in range(8):
                nc.vector.memset(qr2[j][:], 0.0)
            knopeT = [P.tile([128, S], BF16, name=f"knopeT{h}")
                      for h in range(KV)]
            v_sb = [P.tile([128, KV * DV], BF16, name=f"v{m}")
                    for m in range(16)]
            # k_rot^T duplicated in both partition halves so the rope scores
            # matmul can match base_partition with either half of a q pair
            krot2 = P.tile([128, S], BF16, name="krot2")

            # ========================= Q window =========================
            with (
                tc.tile_pool(name="qwin", bufs=1) as QW,
                tc.tile_pool(name="qsc", bufs=2) as QS,
            ):
                # q_a^T [1536, 256] bf16 (raw, pre-rms)
                qaT = [QW.tile([128, TPC], BF16, name=f"qaT{m}")
                       for m in range(12)]
                with tc.tile_pool(name="qaps", bufs=1, space="PSUM") as PSB:
                    for half in range(2):
                        pss = [PSB.tile([128, TPC], F32, name=f"ps_qa{m}",
                                        tag=f"ps_qa{m}") for m in range(6)]
                        for k in range(16):
                            wch = WS.tile([128, 768], BF16, name="wch",
                                          tag="wch")
                            nc.gpsimd.dma_start(
                                wch[:], wqa[k * 128:(k + 1) * 128,
                                            half * 768:(half + 1) * 768])
                            hch = QS.tile([128, TPC], BF16, name="hqch",
                                          tag="hqch", bufs=3)
                            nc.sync.dma_start(
                                hch[:], hTq[k * 128:(k + 1) * 128, :])
                            for m in range(6):
                                nc.tensor.matmul(
                                    pss[m][:], wch[:, m * 128:(m + 1) * 128],
                                    hch[:], start=(k == 0), stop=(k == 15))
                        for m in range(6):
                            nc.vector.tensor_copy(qaT[half * 6 + m][:],
                                                  pss[m][:])

                    # q RMS scale vector (applied at q_b evac: per-token
                    # scaling commutes through the matmul)
                    ps_qss = PSB.tile([1, TPC], F32, name="ps_qss")
                    for m in range(12):
                        sq = QS.tile([128, TPC], BF16, name="sqq", tag="sqq")
                        nc.scalar.activation(
                            sq[:], qaT[m][:],
                            mybir.ActivationFunctionType.Square)
                        nc.tensor.matmul(ps_qss[:], ones_b[:], sq[:],
                                         start=(m == 0), stop=(m == 11))
                    srt_q = QW.tile([1, TPC], F32, name="srt_q")
                    nc.scalar.activation(srt_q[:], ps_qss[:],
                                         mybir.ActivationFunctionType.Sqrt,
                                         bias=eps_sb[:], scale=1.0 / QR)
                    qscale = QW.tile([1, TPC], F32, name="qscale")
                    nc.vector.reciprocal(qscale[:], srt_q[:])
                    nc.sync.dma_start(scr_q[:], qscale[:])
                    qscale_bc = QW.tile([128, TPC], F32, name="qscale_bc")
                    nc.sync.dma_start(qscale_bc[:], bcast_src(scr_q, 0, TPC))

                # q_b: nope per head + rope pairs; rms scale applied at evac
                qrT = [QW.tile([128, TPC], BF16, name=f"qrT{j}")
                       for j in range(8)]
                with tc.tile_pool(name="qbps", bufs=1, space="PSUM") as PSB:
                    for g in range(4):
                        psn = [PSB.tile([128, TPC], F32, name=f"ps_qb{u}",
                                        tag=f"ps_qb{u}") for u in range(6)]
                        for k in range(12):
                            wch = WS.tile([128, 768], BF16, name="wch",
                                          tag="wch")
                            nc.gpsimd.dma_start(
                                wch[:],
                                wqbm[k * 128:(k + 1) * 128,
                                     g * 768:(g + 1) * 768])
                            for l in range(4):
                                nc.tensor.matmul(
                                    psn[l][:], wch[:, l * 128:(l + 1) * 128],
                                    qaT[k][:], start=(k == 0), stop=(k == 11))
                            for lj in range(2):
                                nc.tensor.matmul(
                                    psn[4 + lj][:],
                                    wch[:, 512 + lj * 128:512 + (lj + 1) * 128],
                                    qaT[k][:], start=(k == 0), stop=(k == 11))
                        for l in range(4):
                            nc.vector.tensor_mul(
                                strided(qnT2[2 * g + l // 2][:], l % 2, 2,
                                        TPC),
                                psn[l][:], qscale_bc[:])
                        for lj in range(2):
                            nc.vector.tensor_mul(qrT[g * 2 + lj][:],
                                                 psn[4 + lj][:],
                                                 qscale_bc[:])

                # RoPE on q pairs (rows 0-63 head 2j, 64-127 head 2j+1).
                # out = x*cos2 + rot(x)*sin2 with rot = partition rotate by
                # 32 within each 64-row block (via sbuf->sbuf DMA) and the
                # rotate_half sign folded into sin2 host-side.
                cosq_sb = QW.tile([128, TPC], BF16, name="cosq_sb")
                sinq_sb = QW.tile([128, TPC], BF16, name="sinq_sb")
                nc.sync.dma_start(cosq_sb[:], cosq2[:, :])
                nc.sync.dma_start(sinq_sb[:], sinq2[:, :])
                for j in range(8):
                    xr = QS.tile([128, TPC], BF16, name="xr", tag="xr")
                    for b0, b1 in ((0, 32), (32, 0), (64, 96), (96, 64)):
                        nc.sync.dma_start(xr[b0:b0 + 32, :],
                                          qrT[j][b1:b1 + 32, :])
                    t1 = QS.tile([128, TPC], F32, name="t1q", tag="t1q")
                    nc.vector.tensor_mul(t1[:], qrT[j][:], cosq_sb[:])
                    nc.vector.tensor_mul(xr[:], xr[:], sinq_sb[:])
                    # interleave into qr2: head 2j -> rows 0:64 even cols,
                    # head 2j+1 -> rows 64:128 odd cols
                    nc.vector.tensor_add(
                        strided(qr2[j][0:64, :], 0, 2, TPC),
                        t1[0:64, :], xr[0:64, :])
                    nc.vector.tensor_add(
                        strided(qr2[j][64:128, :], 1, 2, TPC),
                        t1[64:128, :], xr[64:128, :])

            # ========================= KV window =========================
            # fully chunked over 4 token chunks of 512: a-proj -> rms ->
            # rope -> k_nope^T -> v, per chunk.
            with (
                tc.tile_pool(name="kvwin", bufs=1) as KW,
                tc.tile_pool(name="kvch", bufs=2) as KC,
                tc.tile_pool(name="ksc", bufs=2) as KS,
                tc.tile_pool(name="kps", bufs=1, space="PSUM") as PSB,
            ):
                wkva_sb = [KW.tile([128, KVR + DR], BF16, name=f"wkva{k}")
                           for k in range(16)]
                for k in range(16):
                    nc.sync.dma_start(wkva_sb[k][:],
                                      wkva[k * 128:(k + 1) * 128, :])
                wkvbk_sb = [KW.tile([128, KV * DN], BF16, name=f"wkvbk{k}")
                            for k in range(4)]
                wkvbv_sb = [KW.tile([128, KV * DV], BF16, name=f"wkvbv{k}")
                            for k in range(4)]
                for k in range(4):
                    nc.sync.dma_start(wkvbk_sb[k][:],
                                      wkvbk[k * 128:(k + 1) * 128, :])
                    nc.sync.dma_start(wkvbv_sb[k][:],
                                      wkvbv[k * 128:(k + 1) * 128, :])

                m_sizes = [128, 128, 128, 128, 64]
                for n in range(4):
                    ncols = slice(n * 512, (n + 1) * 512)
                    # ---- a-projection for this chunk ----
                    ckv = [KC.tile([m_sizes[m], 512], BF16 if m < 4 else F32R,
                                   name=f"ckv{m}", tag=f"ckv{m}")
                           for m in range(5)]
                    pss = [PSB.tile([m_sizes[m], 512], F32, name=f"ps_kva{m}",
                                    tag=f"ps_kva{m}") for m in range(5)]
                    for k in range(16):
                        hch = WS.tile([128, 768], BF16, name="wch", tag="wch")
                        nc.gpsimd.dma_start(hch[:, 0:512],
                                            hT[k * 128:(k + 1) * 128, ncols])
                        for m in range(5):
                            nc.tensor.matmul(
                                pss[m][:],
                                wkva_sb[k][:, m * 128: m * 128 + m_sizes[m]],
                                hch[:, 0:512], start=(k == 0), stop=(k == 15))
                    for m in range(5):
                        nc.vector.tensor_copy(ckv[m][:], pss[m][:])

                    # ---- RoPE on k_rot (raw; no rms on the rope part):
                    # out = x*cos + rot(x)*sin_signed, rot via DMA ----
                    cos_t = KS.tile([64, 512], F32, name="cos_t", tag="cos_t",
                                    bufs=2)
                    sin_t = KS.tile([64, 512], F32, name="sin_t", tag="sin_t",
                                    bufs=2)
                    nc.sync.dma_start(cos_t[:], cossinT[0:64, ncols])
                    nc.sync.dma_start(sin_t[:], cossinT[64:128, ncols])
                    kxr = KS.tile([64, 512], F32R, name="kxr", tag="kxr")
                    nc.sync.dma_start(kxr[0:32, :], ckv[4][32:64, :])
                    nc.sync.dma_start(kxr[32:64, :], ckv[4][0:32, :])
                    kt1 = KS.tile([64, 512], F32, name="kt1", tag="kt1")
                    nc.vector.tensor_mul(kt1[:], ckv[4][:], cos_t[:])
                    nc.vector.tensor_mul(kxr[:], kxr[:], sin_t[:])
                    nc.vector.tensor_add(krot2[0:64, ncols], kt1[:], kxr[:])
                    # duplicate into the upper partition half
                    nc.sync.dma_start(krot2[64:128, ncols],
                                      krot2[0:64, ncols])

                    # ---- RMS scale vector for this chunk ----
                    ps_ss = PSB.tile([1, 512], F32, name="ps_ssk",
                                     tag="ps_ssk")
                    for m in range(4):
                        sq = KS.tile([128, 512], BF16, name="sqk", tag="sqk")
                        nc.scalar.activation(
                            sq[:], ckv[m][:],
                            mybir.ActivationFunctionType.Square)
                        nc.tensor.matmul(ps_ss[:], ones_b[:], sq[:],
                                         start=(m == 0), stop=(m == 3))
                    srt_k = KS.tile([1, 512], F32, name="srt_k", tag="srt_k")
                    nc.scalar.activation(srt_k[:], ps_ss[:],
                                         mybir.ActivationFunctionType.Sqrt,
                                         bias=eps_sb[:], scale=1.0 / KVR)
                    kscale = KS.tile([1, 512], F32, name="kscale",
                                     tag="kscale")
                    nc.vector.reciprocal(kscale[:], srt_k[:])
                    nc.sync.dma_start(scr_k[:, ncols], kscale[:])
                    kscale_bc = KS.tile([128, 512], F32, name="kscale_bc",
                                        tag="kscale_bc")
                    nc.sync.dma_start(kscale_bc[:],
                                      bcast_src(scr_k, n * 512, 512))
                    # token-partition-shaped scale for v evac:
                    # [p, m] <-> token 128m + p of this chunk
                    kscaleT = KS.tile([128, 4], F32, name="kscaleT",
                                      tag="kscaleT")
                    skap = scr_k[:]
                    nc.sync.dma_start(
                        kscaleT[:],
                        bass.AP(tensor=skap.tensor,
                                offset=skap.offset + n * 512,
                                ap=[[1, 128], [128, 4]]))

                    # ---- k_nope^T for this chunk (rms scale at evac) ----
                    for h in range(KV):
                        ps = PSB.tile([128, 512], F32, name="ps_kn",
                                      tag="ps_kn")
                        for k in range(4):
                            nc.tensor.matmul(
                                ps[:], wkvbk_sb[k][:, h * 128:(h + 1) * 128],
                                ckv[k][:], start=(k == 0), stop=(k == 3))
                        nc.vector.tensor_mul(knopeT[h][:, ncols], ps[:],
                                             kscale_bc[:])

                    # ---- v natural for this chunk's 4 token tiles ----
                    for mm in range(4):
                        ps = PSB.tile([128, 512], F32, name="ps_v", tag="ps_v")
                        for k in range(4):
                            nc.tensor.matmul(
                                ps[:], ckv[k][:, mm * 128:(mm + 1) * 128],
                                wkvbv_sb[k][:], start=(k == 0), stop=(k == 3))
                        nc.vector.tensor_scalar_mul(v_sb[n * 4 + mm][:],
                                                    ps[:],
                                                    kscaleT[:, mm:mm + 1])

            # =========================== Attention ==========================
            # Head-pair packing: pair j = heads (2j, 2j+1); packed column
            # 2q + p <-> (query q, head 2j+p). One nope-score / exp / sum /
            # AV instruction covers both heads (N up to 512); rope scores
            # stay per-head (stride-2 PSUM dst). Causal slicing at 32-query
            # granularity: key block kb only needs queries >= 32*(kb//2).
            attn_T = [P.tile([128, TPC], BF16, name=f"attnT{h}")
                      for h in range(H)]

            with (
                tc.tile_pool(name="aps", bufs=2, space="PSUM") as PSA,
                tc.tile_pool(name="recb", bufs=4) as RB,
            ):
                for j in range(8):
                    hk = j // 2          # kv head for this pair
                    ps_av = PSA.tile([128, 2 * TPC], F32, name="ps_av",
                                     tag="ps_av")
                    ps_sum = PSA.tile([1, 2 * TPC], F32, name="ps_sum",
                                      tag="ps_sum")
                    for kb in range(NB):
                        c0 = 32 * (kb // 2)
                        nq = TPC - c0
                        psl = slice(2 * c0, 2 * TPC)
                        kcols = slice(kb * 128, (kb + 1) * 128)
                        ps_sc = PSA.tile([128, 2 * TPC], F32, name="ps_sc",
                                         tag="ps_sc")
                        nc.tensor.matmul(ps_sc[:, psl], knopeT[hk][:, kcols],
                                         qnT2[j][:, psl], start=True,
                                         stop=False)
                        nc.tensor.matmul(ps_sc[:, psl], krot2[:, kcols],
                                         qr2[j][:, psl],
                                         start=False, stop=True)
                        nc.vector.tensor_add(ps_sc[:, 2 * c0:2 * c0 + 64],
                                             ps_sc[:, 2 * c0:2 * c0 + 64],
                                             bmask_sb[:, kb, :])
                        p_t = PP.tile([128, 2 * TPC], BF16, name="p_t",
                                      tag="p_t")
                        nc.scalar.activation(
                            p_t[:, psl], ps_sc[:, psl],
                            mybir.ActivationFunctionType.Exp, scale=SCALE)
                        nc.tensor.matmul(ps_sum[:, psl], ones_b[:],
                                         p_t[:, psl], start=(kb == 0),
                                         stop=(kb == NB - 1))
                        nc.tensor.matmul(
                            ps_av[:, psl],
                            v_sb[kb][:, hk * 128:(hk + 1) * 128],
                            p_t[:, psl], start=(kb == 0), stop=(kb == NB - 1))
                    rec_t = PP.tile([1, 2 * TPC], F32, name="rec_t",
                                    tag="rec_t", bufs=3)
                    nc.vector.reciprocal(rec_t[:], ps_sum[:])
                    nc.sync.dma_start(scr_r[j:j + 1, :], rec_t[:])
                    # normalize + de-interleave into per-head attn_T
                    srp = scr_r[:]
                    for p in range(2):
                        rb = RB.tile([128, TPC], F32, name="rb", tag="rb")
                        nc.sync.dma_start(rb[:], bass.AP(
                            tensor=srp.tensor,
                            offset=srp.offset + j * 2 * TPC + p,
                            ap=[[0, 128], [2, TPC]]))
                        nc.vector.tensor_mul(attn_T[2 * j + p][:],
                                             strided(ps_av[:], p, 2, TPC),
                                             rb[:])

            # ============================ o_proj ============================
            with (
                tc.tile_pool(name="ops", bufs=1, space="PSUM") as PSB,
                tc.tile_pool(name="wop", bufs=3) as WO,
            ):
                pso = [PSB.tile([128, 512], F32, name=f"ps_o{i}")
                       for i in range(8)]
                for h in range(H):
                    wos = WO.tile([128, 2048], BF16, name="wos", tag="wos")
                    nc.gpsimd.dma_start(wos[:], wo_t[h * 128:(h + 1) * 128, :])
                    for n in range(4):
                        for m in range(2):
                            nc.tensor.matmul(
                                pso[n * 2 + m][:],
                                attn_T[h][:, m * 128:(m + 1) * 128],
                                wos[:, n * 512:(n + 1) * 512],
                                start=(h == 0), stop=(h == H - 1))
                for i in range(8):
                    n, m = i // 2, i % 2
                    osb = PP.tile([128, 512], F32, name="osb", tag="osb",
                                  bufs=2)
                    nc.vector.tensor_copy(osb[:], pso[i][:])
                    nc.sync.dma_start(
                        out[m * 128:(m + 1) * 128, n * 512:(n + 1) * 512],
                        osb[:])

    return nc


def kernel(hidden_states, cos, sin, wq_a, q_a_ln_w, wq_b, wkv_a, kv_a_ln_w,
           wkv_b, wo, cache_position, _trace=False):
    global _BUILT, LAST_RESULTS
    hidden_states = np.asarray(hidden_states, dtype=np.float32)
    cos = np.asarray(cos, dtype=np.float32)
    sin = np.asarray(sin, dtype=np.float32)
    wq_a = np.asarray(wq_a, dtype=np.float32)
    q_a_ln_w = np.asarray(q_a_ln_w, dtype=np.float32)
    wq_b = np.asarray(wq_b, dtype=np.float32)
    wkv_a = np.asarray(wkv_a, dtype=np.float32)
    kv_a_ln_w = np.asarray(kv_a_ln_w, dtype=np.float32)
    wkv_b = np.asarray(wkv_b, dtype=np.float32)
    wo = np.asarray(wo, dtype=np.float32)
    cp = np.asarray(cache_position).astype(np.int64)

    # ---- host-side prep (layout/sharding only) ----
    bf16 = ml_dtypes.bfloat16
    h = hidden_states[0]                       # [S, D]
    hT = np.ascontiguousarray(h.T).astype(bf16)  # [D, S]
    cos_sel = cos[0][cp]                       # [S, DR]
    sin_sel = sin[0][cp]
    cosT = np.ascontiguousarray(cos_sel.T)     # [DR, S]
    sinT = np.ascontiguousarray(sin_sel.T)
    # fold the rmsnorm elementwise weights into the b-projections
    wqb_eff = wq_b * q_a_ln_w[:, None]
    wqb_r3 = wqb_eff.reshape(QR, H, DQK)
    wqbn = wqb_r3[:, :, :DN].reshape(QR, H * DN)
    wqbr = wqb_r3[:, :, DN:].reshape(QR, H * DR)
    # merged per-group layout: [512 nope | 256 rope] x 4 groups
    wqbm = np.empty((QR, H * DQK), np.float32)
    for g in range(4):
        wqbm[:, g * 768:g * 768 + 512] = wqbn[:, g * 512:(g + 1) * 512]
        wqbm[:, g * 768 + 512:(g + 1) * 768] = wqbr[:, g * 256:(g + 1) * 256]
    wqbm = wqbm.astype(bf16)
    wkvb_eff = wkv_b * kv_a_ln_w[:, None]      # [KVR, KV*(DN+DV)]
    wkvb_r = wkvb_eff.reshape(KVR, KV, DN + DV)
    wkvbk = np.ascontiguousarray(wkvb_r[:, :, :DN].reshape(KVR, KV * DN)).astype(bf16)
    wkvbv = np.ascontiguousarray(wkvb_r[:, :, DN:].reshape(KVR, KV * DV)).astype(bf16)
    wo_c = np.ascontiguousarray(wo).astype(bf16)
    wqa_c = wq_a.astype(bf16)
    wkva_c = wkv_a.astype(bf16)

    sgn = np.concatenate([-np.ones(DR // 2), np.ones(DR // 2)]
                         ).astype(np.float32)[:, None]
    cossinT = np.ascontiguousarray(
        np.concatenate([cosT, sinT * sgn], axis=0))
    in_maps = []
    for c in range(NC_):
        toks = np.arange(c, S, NC_)            # this core's 256 tokens
        hTq = np.ascontiguousarray(hT[:, toks])
        cq = cosT[:, toks]
        sq = (sinT * sgn)[:, toks]
        cosq2 = np.ascontiguousarray(
            np.concatenate([cq, cq], axis=0)).astype(bf16)
        sinq2 = np.ascontiguousarray(
            np.concatenate([sq, sq], axis=0)).astype(bf16)
        # band masks over the diagonal 64 interleaved packed columns of key
        # block kb: column 2*(q - c0) + p <-> query q, c0 = 32*(kb//2);
        # bm[kb][k, i] = 0 if key 128kb+k <= token 8q+c else NEG
        r_ = np.arange(16)[:, None, None]
        k_ = np.arange(128)[None, :, None]
        i_ = np.arange(64)[None, None, :]
        q_ = 32 * (r_ // 2) + i_ // 2
        bm = np.where(128 * r_ + k_ <= 8 * q_ + c, 0.0, NEG).astype(np.float32)
        bm_dev = np.ascontiguousarray(
            bm.transpose(1, 0, 2).reshape(128, 16 * 64))
        in_maps.append({
            "hT": hT, "hTq": hTq, "wqa": wqa_c, "wqbm": wqbm,
            "wkva": wkva_c, "wkvbk": wkvbk, "wkvbv": wkvbv, "wo": wo_c,
            "cossinT": cossinT, "cosq2": cosq2, "sinq2": sinq2,
            "bmask": bm_dev,
        })

    if _BUILT is None:
        _BUILT = _build()
    nc = _BUILT

    res = run_bass_kernel_spmd(nc, in_maps, core_ids=list(range(NC_)),
                               trace=_trace)
    LAST_RESULTS = res

    out_full = np.empty((S, D), dtype=np.float32)
    for c in range(NC_):
        out_full[c::NC_] = res.results[c]["out"]   # row m <-> token 8m+c
    return out_full[None]                      # [1, S, D]



# revision 35
# speedup vs baseline: 2.9335x; 2.9335x over previous
"""DeepseekV3 attention (B=1, S=2048, D=2048, H=16, KV=4) on 8 trn2 cores.

Sharding: token-modulo-8 split. Core c owns query tokens {t : t % 8 == c}
(256 each) — causal attention work is identical on every core, so one SPMD
program serves all 8 cores with per-core DATA (host-sliced hidden columns,
cos/sin slices, causal band masks) carrying the differences.

Per core:
  - kv path replicated: ckv^T = wkv_a^T @ h^T over all 2048 tokens, RMS,
    k_nope^T / v via wkv_b, RoPE on k_rot.
  - q path token-split: q_a^T/q^T only for the core's 256 tokens.
  - attention: scores computed transposed [k, q] (lhsT = k^T tiles), softmax
    denominator via ones-matmul, AV with v in natural [token, dv] layout.
    Causal masking: additive band masks (input data) on the diagonal bands.
  - o_proj over the core's 256 token rows; host reassembles rows.

All matmuls run in bf16 (1 cycle/row on the PE at any free size, vs fp32r's
2-4 and heavier power throttling) with fp32 PSUM accumulation; weights and
hidden states are cast to bf16 host-side, which also halves HBM traffic.
"""
import math
import sys
import types

import ml_dtypes
import numpy as np

# ---------------------------------------------------------------------------
# Container compat: this walrus build rejects instructions carrying more than
# one sync-wait command. Patch Tile to (a) split multi-wait instructions into
# single-wait NoOps on the same engine, (b) hoist the end-of-kernel drain's
# waits onto single-wait NOPs. Also register the NTFF profile hook (the
# image's antenv lacks axon_hooks) so trace=True works for profiling.
# ---------------------------------------------------------------------------
import concourse.bass as bass
import concourse.mybir as mybir
import concourse.tile as tile
from concourse.bass_utils import run_bass_kernel_spmd
from concourse.tile import ScopedClock
from bass_rust import VectorClock

N_PROCS = len(VectorClock())
_PATCHED = False


def _install_ntff_hook():
    if 'antenv.axon_hooks' in sys.modules:
        return
    m = types.ModuleType('antenv.axon_hooks')
    holder = [None]
    m.set_axon_ntff_profile_hook = lambda h: holder.__setitem__(0, h)
    m.get_axon_ntff_profile_hook = lambda: holder[0]
    sys.modules['antenv.axon_hooks'] = m
    try:
        from trn_agent_boot.trn_boot import _ntff_profile_via_ctypes
        m.set_axon_ntff_profile_hook(
            _ntff_profile_via_ctypes('/opt/axon/libaxon_pjrt.so'))
    except Exception:
        pass


def _patched_drain_and_barrier(self, tick_clock, wait_clock):
    gc = tick_clock.global_clock
    for p in range(N_PROCS):
        if gc[p] == 0:
            continue
        single = VectorClock([gc[q] if q == p else 0 for q in range(N_PROCS)])
        nop_inst = self.nc.sync.nop(nofuse=True)
        wait_clock.add_sem_waits(nop_inst.ins, ScopedClock({None: single}))
    self.nc.sync.drain()
    self.nc.all_engine_barrier()
    popped = self.nc._tile_sem_poison_stack.pop()
    assert popped is self._sem_poison
    self.nc.clear_and_free_semaphores(list(self.sems.allocated().values()))
    self.nc.all_engine_barrier()


def _make_split_lower(orig):
    def _split_multi_waits(self, ordered):
        nc = self.nc
        for bb_name, insts in ordered.items():
            out = []
            for inst in insts:
                si = inst.sync_info
                waits = list(si.on_wait) if si is not None else []
                if len(waits) > 1:
                    for w in waits[:-1]:
                        nop = mybir.InstNoOp(
                            name=f"{inst.name}-waitsplit-{nc.next_id()}",
                            engine=inst.engine,
                            sync_info=mybir.SyncInfo(on_wait=[w], on_update=[]),
                        )
                        nc.register_instruction(nop)
                        out.append(nop)
                    inst.sync_info = mybir.SyncInfo(
                        on_wait=[waits[-1]], on_update=list(si.on_update))
                out.append(inst)
            ordered[bb_name] = out
        return orig(self, ordered)
    return _split_multi_waits


def _install_patches():
    global _PATCHED
    _install_ntff_hook()
    if _PATCHED:
        return
    tile.TileContext._drain_and_barrier = _patched_drain_and_barrier
    tile.TileContext._lower_ordered_insts = _make_split_lower(
        tile.TileContext._lower_ordered_insts)
    _PATCHED = True


_install_patches()

# ---------------------------------------------------------------------------
# Problem constants (hardcoded per the spec).
# ---------------------------------------------------------------------------
S = 2048
D = 2048
H = 16
KV = 4
GROUPS = H // KV
DN = 128          # d_nope
DR = 64           # d_rope
DQK = DN + DR     # 192
DV = 128
QR = 1536         # q rank
KVR = 512         # kv rank
EPS = 1e-6
NC_ = 8           # cores
TPC = S // NC_    # 256 tokens per core
NB = S // 128     # 16 k-subtiles
SCALE = 1.0 / math.sqrt(DQK)
NEG = -1e30

F32 = mybir.dt.float32
F32R = mybir.dt.float32r
BF16 = mybir.dt.bfloat16

_BUILT = None     # cached (nc,) so repeat kernel() calls skip rebuild
LAST_RESULTS = None  # BassKernelResults stash for test.py


def _build():
    nc = bass.Bass()

    # ---- DRAM I/O (identical declaration on all cores; data differs) ----
    hT = nc.dram_tensor("hT", [D, S], BF16, kind="ExternalInput")
    hTq = nc.dram_tensor("hTq", [D, TPC], BF16, kind="ExternalInput")
    wqa = nc.dram_tensor("wqa", [D, QR], BF16, kind="ExternalInput")
    wqbm = nc.dram_tensor("wqbm", [QR, H * DQK], BF16, kind="ExternalInput")
    wkva = nc.dram_tensor("wkva", [D, KVR + DR], BF16, kind="ExternalInput")
    wkvbk = nc.dram_tensor("wkvbk", [KVR, KV * DN], BF16, kind="ExternalInput")
    wkvbv = nc.dram_tensor("wkvbv", [KVR, KV * DV], BF16, kind="ExternalInput")
    wo_t = nc.dram_tensor("wo", [H * DV, D], BF16, kind="ExternalInput")
    cossinT = nc.dram_tensor("cossinT", [2 * DR, S], F32, kind="ExternalInput")
    cosq2 = nc.dram_tensor("cosq2", [2 * DR, TPC], BF16, kind="ExternalInput")
    sinq2 = nc.dram_tensor("sinq2", [2 * DR, TPC], BF16, kind="ExternalInput")
    bmask = nc.dram_tensor("bmask", [128, 16 * 64], F32, kind="ExternalInput")
    out = nc.dram_tensor("out", [TPC, D], F32, kind="ExternalOutput")
    # scratch for free->partition broadcasts
    scr_k = nc.dram_tensor("scr_k", [1, S], F32, kind="Internal")
    scr_q = nc.dram_tensor("scr_q", [1, TPC], F32, kind="Internal")
    scr_r = nc.dram_tensor("scr_r", [8, 2 * TPC], F32, kind="Internal")

    def bcast_src(dram, off, ncols):
        # element off.. of the flat DRAM vector, broadcast to 128 partitions
        ap = dram[:]
        return bass.AP(tensor=ap.tensor, offset=ap.offset + off,
                       ap=[[0, 128], [1, ncols]])

    def strided(ap_src, start, stride, count):
        # free-dim strided view of a full 2D sbuf/psum tile AP
        return bass.AP(tensor=ap_src.tensor, offset=ap_src.offset + start,
                       ap=[ap_src.ap[0], [stride, count]])

    with tile.TileContext(nc) as tc:
        with (
            tc.tile_pool(name="persist", bufs=1) as P,   # attention-lived
            tc.tile_pool(name="wstream", bufs=3) as WS,  # streamed weights
            tc.tile_pool(name="ppool", bufs=6) as PP,    # p tiles (bf16)
        ):
            ones_b = P.tile([128, 1], BF16, name="ones_b")
            nc.vector.memset(ones_b[:], 1.0)
            eps_sb = P.tile([1, 1], F32, name="eps_sb")
            nc.vector.memset(eps_sb[:], EPS)
            # bmask[k, kb, i]: causal band masks over the 64 interleaved
            # packed columns at the diagonal of key block kb
            bmask_sb = P.tile([128, 16, 64], F32, name="bmask_sb")
            nc.sync.dma_start(bmask_sb[:], bmask[:, :].rearrange(
                "k (r i) -> k r i", r=16))

            # attention-lived products; qnT2[j] holds the pair (2j, 2j+1)
            # nope queries interleaved: col 2q+p <-> (query q, head 2j+p)
            qnT2 = [P.tile([128, 2 * TPC], BF16, name=f"qnT2{j}")
                    for j in range(8)]
            # qr2[j]: roped queries, zero-padded interleave — rows 0:64 hold
            # head 2j at even cols, rows 64:128 head 2j+1 at odd cols, so one
            # matmul against the duplicated krot2 yields both heads' rope
            # scores at the packed columns.
            qr2 = [P.tile([128, 2 * TPC], BF16, name=f"qr2{j}")
                   for j in range(8)]
            for j in range(8):
                nc.vector.memset(qr2[j][:], 0.0)
            knopeT = [P.tile([128, S], BF16, name=f"knopeT{h}")
                      for h in range(KV)]
            v_sb = [P.tile([128, KV * DV], BF16, name=f"v{m}")
                    for m in range(16)]
            # k_rot^T duplicated in both partition halves so the rope scores
            # matmul can match base_partition with either half of a q pair
            krot2 = P.tile([128, S], BF16, name="krot2")

            # ========================= Q window =========================
            with (
                tc.tile_pool(name="qwin", bufs=1) as QW,
                tc.tile_pool(name="qsc", bufs=2) as QS,
            ):
                # q_a^T [1536, 256] bf16 (raw, pre-rms)
                qaT = [QW.tile([128, TPC], BF16, name=f"qaT{m}")
                       for m in range(12)]
                with tc.tile_pool(name="qaps", bufs=1, space="PSUM") as PSB:
                    for half in range(2):
                        pss = [PSB.tile([128, TPC], F32, name=f"ps_qa{m}",
                                        tag=f"ps_qa{m}") for m in range(6)]
                        for k in range(16):
                            wch = WS.tile([128, 768], BF16, name="wch",
                                          tag="wch")
                            nc.gpsimd.dma_start(
                                wch[:], wqa[k * 128:(k + 1) * 128,
                                            half * 768:(half + 1) * 768])
                            hch = QS.tile([128, TPC], BF16, name="hqch",
                                          tag="hqch", bufs=3)
                            nc.sync.dma_start(
                                hch[:], hTq[k * 128:(k + 1) * 128, :])
                            for m in range(6):
                                nc.tensor.matmul(
                                    pss[m][:], wch[:, m * 128:(m + 1) * 128],
                                    hch[:], start=(k == 0), stop=(k == 15))
                        for m in range(6):
                            nc.vector.tensor_copy(qaT[half * 6 + m][:],
                                                  pss[m][:])

                    # q RMS scale vector (applied at q_b evac: per-token
                    # scaling commutes through the matmul)
                    ps_qss = PSB.tile([1, TPC], F32, name="ps_qss")
                    for m in range(12):
                        sq = QS.tile([128, TPC], BF16, name="sqq", tag="sqq")
                        nc.scalar.activation(
                            sq[:], qaT[m][:],
                            mybir.ActivationFunctionType.Square)
                        nc.tensor.matmul(ps_qss[:], ones_b[:], sq[:],
                                         start=(m == 0), stop=(m == 11))
                    srt_q = QW.tile([1, TPC], F32, name="srt_q")
                    nc.scalar.activation(srt_q[:], ps_qss[:],
                                         mybir.ActivationFunctionType.Sqrt,
                                         bias=eps_sb[:], scale=1.0 / QR)
                    nc.sync.dma_start(scr_q[:], srt_q[:])
                    qsb = QW.tile([128, TPC], F32, name="qsb")
                    nc.sync.dma_start(qsb[:], bcast_src(scr_q, 0, TPC))
                    qscale_bc = QW.tile([128, TPC], F32, name="qscale_bc")
                    nc.vector.reciprocal(qscale_bc[:], qsb[:])

                # q_b: nope per head + rope pairs; rms scale applied at evac
                qrT = [QW.tile([128, TPC], BF16, name=f"qrT{j}")
                       for j in range(8)]
                with tc.tile_pool(name="qbps", bufs=1, space="PSUM") as PSB:
                    for g in range(4):
                        psn = [PSB.tile([128, TPC], F32, name=f"ps_qb{u}",
                                        tag=f"ps_qb{u}") for u in range(6)]
                        for k in range(12):
                            wch = WS.tile([128, 768], BF16, name="wch",
                                          tag="wch")
                            nc.gpsimd.dma_start(
                                wch[:],
                                wqbm[k * 128:(k + 1) * 128,
                                     g * 768:(g + 1) * 768])
                            for l in range(4):
                                nc.tensor.matmul(
                                    psn[l][:], wch[:, l * 128:(l + 1) * 128],
                                    qaT[k][:], start=(k == 0), stop=(k == 11))
                            for lj in range(2):
                                nc.tensor.matmul(
                                    psn[4 + lj][:],
                                    wch[:, 512 + lj * 128:512 + (lj + 1) * 128],
                                    qaT[k][:], start=(k == 0), stop=(k == 11))
                        for l in range(4):
                            nc.vector.tensor_mul(
                                strided(qnT2[2 * g + l // 2][:], l % 2, 2,
                                        TPC),
                                psn[l][:], qscale_bc[:])
                        for lj in range(2):
                            nc.vector.tensor_mul(qrT[g * 2 + lj][:],
                                                 psn[4 + lj][:],
                                                 qscale_bc[:])

                # RoPE on q pairs (rows 0-63 head 2j, 64-127 head 2j+1).
                # out = x*cos2 + rot(x)*sin2 with rot = partition rotate by
                # 32 within each 64-row block (via sbuf->sbuf DMA) and the
                # rotate_half sign folded into sin2 host-side.
                cosq_sb = QW.tile([128, TPC], BF16, name="cosq_sb")
                sinq_sb = QW.tile([128, TPC], BF16, name="sinq_sb")
                nc.sync.dma_start(cosq_sb[:], cosq2[:, :])
                nc.sync.dma_start(sinq_sb[:], sinq2[:, :])
                for j in range(8):
                    xr = QS.tile([128, TPC], BF16, name="xr", tag="xr")
                    for b0, b1 in ((0, 32), (32, 0), (64, 96), (96, 64)):
                        nc.sync.dma_start(xr[b0:b0 + 32, :],
                                          qrT[j][b1:b1 + 32, :])
                    t1 = QS.tile([128, TPC], F32, name="t1q", tag="t1q")
                    nc.vector.tensor_mul(t1[:], qrT[j][:], cosq_sb[:])
                    nc.vector.tensor_mul(xr[:], xr[:], sinq_sb[:])
                    # interleave into qr2: head 2j -> rows 0:64 even cols,
                    # head 2j+1 -> rows 64:128 odd cols
                    nc.vector.tensor_add(
                        strided(qr2[j][0:64, :], 0, 2, TPC),
                        t1[0:64, :], xr[0:64, :])
                    nc.vector.tensor_add(
                        strided(qr2[j][64:128, :], 1, 2, TPC),
                        t1[64:128, :], xr[64:128, :])

            # ========================= KV window =========================
            # fully chunked over 4 token chunks of 512: a-proj -> rms ->
            # rope -> k_nope^T -> v, per chunk.
            with (
                tc.tile_pool(name="kvwin", bufs=1) as KW,
                tc.tile_pool(name="kvch", bufs=2) as KC,
                tc.tile_pool(name="ksc", bufs=2) as KS,
                tc.tile_pool(name="kps", bufs=1, space="PSUM") as PSB,
            ):
                wkva_sb = [KW.tile([128, KVR + DR], BF16, name=f"wkva{k}")
                           for k in range(16)]
                for k in range(16):
                    nc.sync.dma_start(wkva_sb[k][:],
                                      wkva[k * 128:(k + 1) * 128, :])
                wkvbk_sb = [KW.tile([128, KV * DN], BF16, name=f"wkvbk{k}")
                            for k in range(4)]
                wkvbv_sb = [KW.tile([128, KV * DV], BF16, name=f"wkvbv{k}")
                            for k in range(4)]
                for k in range(4):
                    nc.sync.dma_start(wkvbk_sb[k][:],
                                      wkvbk[k * 128:(k + 1) * 128, :])
                    nc.sync.dma_start(wkvbv_sb[k][:],
                                      wkvbv[k * 128:(k + 1) * 128, :])

                m_sizes = [128, 128, 128, 128, 64]
                for n in range(4):
                    ncols = slice(n * 512, (n + 1) * 512)
                    # ---- a-projection for this chunk ----
                    ckv = [KC.tile([m_sizes[m], 512], BF16 if m < 4 else F32R,
                                   name=f"ckv{m}", tag=f"ckv{m}")
                           for m in range(5)]
                    pss = [PSB.tile([m_sizes[m], 512], F32, name=f"ps_kva{m}",
                                    tag=f"ps_kva{m}") for m in range(5)]
                    for k in range(16):
                        hch = WS.tile([128, 768], BF16, name="wch", tag="wch")
                        nc.gpsimd.dma_start(hch[:, 0:512],
                                            hT[k * 128:(k + 1) * 128, ncols])
                        for m in range(5):
                            nc.tensor.matmul(
                                pss[m][:],
                                wkva_sb[k][:, m * 128: m * 128 + m_sizes[m]],
                                hch[:, 0:512], start=(k == 0), stop=(k == 15))
                    for m in range(5):
                        nc.vector.tensor_copy(ckv[m][:], pss[m][:])

                    # ---- RoPE on k_rot (raw; no rms on the rope part):
                    # out = x*cos + rot(x)*sin_signed, rot via DMA ----
                    cos_t = KS.tile([64, 512], F32, name="cos_t", tag="cos_t",
                                    bufs=2)
                    sin_t = KS.tile([64, 512], F32, name="sin_t", tag="sin_t",
                                    bufs=2)
                    nc.sync.dma_start(cos_t[:], cossinT[0:64, ncols])
                    nc.sync.dma_start(sin_t[:], cossinT[64:128, ncols])
                    kxr = KS.tile([64, 512], F32R, name="kxr", tag="kxr")
                    nc.sync.dma_start(kxr[0:32, :], ckv[4][32:64, :])
                    nc.sync.dma_start(kxr[32:64, :], ckv[4][0:32, :])
                    kt1 = KS.tile([64, 512], F32, name="kt1", tag="kt1")
                    nc.vector.tensor_mul(kt1[:], ckv[4][:], cos_t[:])
                    nc.vector.tensor_mul(kxr[:], kxr[:], sin_t[:])
                    nc.vector.tensor_add(krot2[0:64, ncols], kt1[:], kxr[:])
                    # duplicate into the upper partition half
                    nc.sync.dma_start(krot2[64:128, ncols],
                                      krot2[0:64, ncols])

                    # ---- RMS scale vector for this chunk ----
                    ps_ss = PSB.tile([1, 512], F32, name="ps_ssk",
                                     tag="ps_ssk")
                    for m in range(4):
                        sq = KS.tile([128, 512], BF16, name="sqk", tag="sqk")
                        nc.scalar.activation(
                            sq[:], ckv[m][:],
                            mybir.ActivationFunctionType.Square)
                        nc.tensor.matmul(ps_ss[:], ones_b[:], sq[:],
                                         start=(m == 0), stop=(m == 3))
                    srt_k = KS.tile([1, 512], F32, name="srt_k", tag="srt_k")
                    nc.scalar.activation(srt_k[:], ps_ss[:],
                                         mybir.ActivationFunctionType.Sqrt,
                                         bias=eps_sb[:], scale=1.0 / KVR)
                    nc.sync.dma_start(scr_k[:, ncols], srt_k[:])
                    ksb = KS.tile([128, 512], F32, name="ksb", tag="ksb")
                    nc.sync.dma_start(ksb[:], bcast_src(scr_k, n * 512, 512))
                    kscale_bc = KS.tile([128, 512], F32, name="kscale_bc",
                                        tag="kscale_bc")
                    nc.vector.reciprocal(kscale_bc[:], ksb[:])
                    # token-partition-shaped scale for v evac:
                    # [p, m] <-> token 128m + p of this chunk
                    kscaleT = KS.tile([128, 4], F32, name="kscaleT",
                                      tag="kscaleT")
                    skap = scr_k[:]
                    nc.sync.dma_start(
                        kscaleT[:],
                        bass.AP(tensor=skap.tensor,
                                offset=skap.offset + n * 512,
                                ap=[[1, 128], [128, 4]]))
                    nc.vector.reciprocal(kscaleT[:], kscaleT[:])

                    # ---- k_nope^T for this chunk (rms scale at evac) ----
                    for h in range(KV):
                        ps = PSB.tile([128, 512], F32, name="ps_kn",
                                      tag="ps_kn")
                        for k in range(4):
                            nc.tensor.matmul(
                                ps[:], wkvbk_sb[k][:, h * 128:(h + 1) * 128],
                                ckv[k][:], start=(k == 0), stop=(k == 3))
                        nc.vector.tensor_mul(knopeT[h][:, ncols], ps[:],
                                             kscale_bc[:])

                    # ---- v natural for this chunk's 4 token tiles ----
                    for mm in range(4):
                        ps = PSB.tile([128, 512], F32, name="ps_v", tag="ps_v")
                        for k in range(4):
                            nc.tensor.matmul(
                                ps[:], ckv[k][:, mm * 128:(mm + 1) * 128],
                                wkvbv_sb[k][:], start=(k == 0), stop=(k == 3))
                        nc.vector.tensor_scalar_mul(v_sb[n * 4 + mm][:],
                                                    ps[:],
                                                    kscaleT[:, mm:mm + 1])

            # =========================== Attention ==========================
            # Head-pair packing: pair j = heads (2j, 2j+1); packed column
            # 2q + p <-> (query q, head 2j+p). One nope-score / exp / sum /
            # AV instruction covers both heads (N up to 512); rope scores
            # stay per-head (stride-2 PSUM dst). Causal slicing at 32-query
            # granularity: key block kb only needs queries >= 32*(kb//2).
            attn_T = [P.tile([128, TPC], BF16, name=f"attnT{h}")
                      for h in range(H)]

            with (
                tc.tile_pool(name="aps", bufs=2, space="PSUM") as PSA,
                tc.tile_pool(name="recb", bufs=4) as RB,
            ):
                for j in range(8):
                    hk = j // 2          # kv head for this pair
                    ps_av = PSA.tile([128, 2 * TPC], F32, name="ps_av",
                                     tag="ps_av")
                    ps_sum = PSA.tile([1, 2 * TPC], F32, name="ps_sum",
                                      tag="ps_sum")
                    for kb in range(NB):
                        c0 = 32 * (kb // 2)
                        nq = TPC - c0
                        psl = slice(2 * c0, 2 * TPC)
                        kcols = slice(kb * 128, (kb + 1) * 128)
                        ps_sc = PSA.tile([128, 2 * TPC], F32, name="ps_sc",
                                         tag="ps_sc")
                        nc.tensor.matmul(ps_sc[:, psl], knopeT[hk][:, kcols],
                                         qnT2[j][:, psl], start=True,
                                         stop=False)
                        nc.tensor.matmul(ps_sc[:, psl], krot2[:, kcols],
                                         qr2[j][:, psl],
                                         start=False, stop=True)
                        nc.vector.tensor_add(ps_sc[:, 2 * c0:2 * c0 + 64],
                                             ps_sc[:, 2 * c0:2 * c0 + 64],
                                             bmask_sb[:, kb, :])
                        p_t = PP.tile([128, 2 * TPC], BF16, name="p_t",
                                      tag="p_t")
                        nc.scalar.activation(
                            p_t[:, psl], ps_sc[:, psl],
                            mybir.ActivationFunctionType.Exp, scale=SCALE)
                        nc.tensor.matmul(ps_sum[:, psl], ones_b[:],
                                         p_t[:, psl], start=(kb == 0),
                                         stop=(kb == NB - 1))
                        nc.tensor.matmul(
                            ps_av[:, psl],
                            v_sb[kb][:, hk * 128:(hk + 1) * 128],
                            p_t[:, psl], start=(kb == 0), stop=(kb == NB - 1))
                    # broadcast raw sums (contiguous DMA), reciprocal on the
                    # broadcast tile (128 lanes), de-interleave on the DVE
                    sum_sb = PP.tile([1, 2 * TPC], F32, name="sum_sb",
                                     tag="sum_sb", bufs=3)
                    nc.vector.tensor_copy(sum_sb[:], ps_sum[:])
                    nc.sync.dma_start(scr_r[j:j + 1, :], sum_sb[:])
                    rbs = RB.tile([128, 2 * TPC], F32, name="rbs", tag="rbs")
                    nc.sync.dma_start(rbs[:],
                                      bcast_src(scr_r, j * 2 * TPC, 2 * TPC))
                    rbr = RB.tile([128, 2 * TPC], F32, name="rbr", tag="rbr")
                    nc.vector.reciprocal(rbr[:], rbs[:])
                    for p in range(2):
                        nc.vector.tensor_mul(attn_T[2 * j + p][:],
                                             strided(ps_av[:], p, 2, TPC),
                                             strided(rbr[:], p, 2, TPC))

            # ============================ o_proj ============================
            with (
                tc.tile_pool(name="ops", bufs=1, space="PSUM") as PSB,
                tc.tile_pool(name="wop", bufs=3) as WO,
            ):
                pso = [PSB.tile([128, 512], F32, name=f"ps_o{i}")
                       for i in range(8)]
                for h in range(H):
                    wos = WO.tile([128, 2048], BF16, name="wos", tag="wos")
                    nc.gpsimd.dma_start(wos[:], wo_t[h * 128:(h + 1) * 128, :])
                    for n in range(4):
                        for m in range(2):
                            nc.tensor.matmul(
                                pso[n * 2 + m][:],
                                attn_T[h][:, m * 128:(m + 1) * 128],
                                wos[:, n * 512:(n + 1) * 512],
                                start=(h == 0), stop=(h == H - 1))
                for i in range(8):
                    n, m = i // 2, i % 2
                    osb = PP.tile([128, 512], F32, name="osb", tag="osb",
                                  bufs=2)
                    nc.vector.tensor_copy(osb[:], pso[i][:])
                    nc.sync.dma_start(
                        out[m * 128:(m + 1) * 128, n * 512:(n + 1) * 512],
                        osb[:])

    return nc


def kernel(hidden_states, cos, sin, wq_a, q_a_ln_w, wq_b, wkv_a, kv_a_ln_w,
           wkv_b, wo, cache_position, _trace=False):
    global _BUILT, LAST_RESULTS
    hidden_states = np.asarray(hidden_states, dtype=np.float32)
    cos = np.asarray(cos, dtype=np.float32)
    sin = np.asarray(sin, dtype=np.float32)
    wq_a = np.asarray(wq_a, dtype=np.float32)
    q_a_ln_w = np.asarray(q_a_ln_w, dtype=np.float32)
    wq_b = np.asarray(wq_b, dtype=np.float32)
    wkv_a = np.asarray(wkv_a, dtype=np.float32)
    kv_a_ln_w = np.asarray(kv_a_ln_w, dtype=np.float32)
    wkv_b = np.asarray(wkv_b, dtype=np.float32)
    wo = np.asarray(wo, dtype=np.float32)
    cp = np.asarray(cache_position).astype(np.int64)

    # ---- host-side prep (layout/sharding only) ----
    bf16 = ml_dtypes.bfloat16
    h = hidden_states[0]                       # [S, D]
    hT = np.ascontiguousarray(h.T).astype(bf16)  # [D, S]
    cos_sel = cos[0][cp]                       # [S, DR]
    sin_sel = sin[0][cp]
    cosT = np.ascontiguousarray(cos_sel.T)     # [DR, S]
    sinT = np.ascontiguousarray(sin_sel.T)
    # fold the rmsnorm elementwise weights into the b-projections
    wqb_eff = wq_b * q_a_ln_w[:, None]
    wqb_r3 = wqb_eff.reshape(QR, H, DQK)
    wqbn = wqb_r3[:, :, :DN].reshape(QR, H * DN)
    wqbr = wqb_r3[:, :, DN:].reshape(QR, H * DR)
    # merged per-group layout: [512 nope | 256 rope] x 4 groups
    wqbm = np.empty((QR, H * DQK), np.float32)
    for g in range(4):
        wqbm[:, g * 768:g * 768 + 512] = wqbn[:, g * 512:(g + 1) * 512]
        wqbm[:, g * 768 + 512:(g + 1) * 768] = wqbr[:, g * 256:(g + 1) * 256]
    wqbm = wqbm.astype(bf16)
    wkvb_eff = wkv_b * kv_a_ln_w[:, None]      # [KVR, KV*(DN+DV)]
    wkvb_r = wkvb_eff.reshape(KVR, KV, DN + DV)
    wkvbk = np.ascontiguousarray(wkvb_r[:, :, :DN].reshape(KVR, KV * DN)).astype(bf16)
    wkvbv = np.ascontiguousarray(wkvb_r[:, :, DN:].reshape(KVR, KV * DV)).astype(bf16)
    wo_c = np.ascontiguousarray(wo).astype(bf16)
    wqa_c = wq_a.astype(bf16)
    wkva_c = wkv_a.astype(bf16)

    sgn = np.concatenate([-np.ones(DR // 2), np.ones(DR // 2)]
                         ).astype(np.float32)[:, None]
    cossinT = np.ascontiguousarray(
        np.concatenate([cosT, sinT * sgn], axis=0))
    in_maps = []
    for c in range(NC_):
        toks = np.arange(c, S, NC_)            # this core's 256 tokens
        hTq = np.ascontiguousarray(hT[:, toks])
        cq = cosT[:, toks]
        sq = (sinT * sgn)[:, toks]
        cosq2 = np.ascontiguousarray(
            np.concatenate([cq, cq], axis=0)).astype(bf16)
        sinq2 = np.ascontiguousarray(
            np.concatenate([sq, sq], axis=0)).astype(bf16)
        # band masks over the diagonal 64 interleaved packed columns of key
        # block kb: column 2*(q - c0) + p <-> query q, c0 = 32*(kb//2);
        # bm[kb][k, i] = 0 if key 128kb+k <= token 8q+c else NEG
        r_ = np.arange(16)[:, None, None]
        k_ = np.arange(128)[None, :, None]
        i_ = np.arange(64)[None, None, :]
        q_ = 32 * (r_ // 2) + i_ // 2
        bm = np.where(128 * r_ + k_ <= 8 * q_ + c, 0.0, NEG).astype(np.float32)
        bm_dev = np.ascontiguousarray(
            bm.transpose(1, 0, 2).reshape(128, 16 * 64))
        in_maps.append({
            "hT": hT, "hTq": hTq, "wqa": wqa_c, "wqbm": wqbm,
            "wkva": wkva_c, "wkvbk": wkvbk, "wkvbv": wkvbv, "wo": wo_c,
            "cossinT": cossinT, "cosq2": cosq2, "sinq2": sinq2,
            "bmask": bm_dev,
        })

    if _BUILT is None:
        _BUILT = _build()
    nc = _BUILT

    res = run_bass_kernel_spmd(nc, in_maps, core_ids=list(range(NC_)),
                               trace=_trace)
    LAST_RESULTS = res

    out_full = np.empty((S, D), dtype=np.float32)
    for c in range(NC_):
        out_full[c::NC_] = res.results[c]["out"]   # row m <-> token 8m+c
    return out_full[None]                      # [1, S, D]



# revision 44
# speedup vs baseline: 3.4795x; 1.1861x over previous
"""DeepseekV3 attention (B=1, S=2048, D=2048, H=16, KV=4) on 8 trn2 cores.

Sharding: token-modulo-8 split. Core c owns query tokens {t : t % 8 == c}
(256 each) — causal attention work is identical on every core, so one SPMD
program serves all 8 cores with per-core DATA (host-sliced hidden columns,
cos/sin slices, causal band masks) carrying the differences.

Per core:
  - kv path replicated: ckv^T = wkv_a^T @ h^T over all 2048 tokens, RMS,
    k_nope^T / v via wkv_b, RoPE on k_rot.
  - q path token-split: q_a^T/q^T only for the core's 256 tokens.
  - attention: scores computed transposed [k, q] (lhsT = k^T tiles), softmax
    denominator via ones-matmul, AV with v in natural [token, dv] layout.
    Causal masking: additive band masks (input data) on the diagonal bands.
  - o_proj over the core's 256 token rows; host reassembles rows.

All matmuls run in bf16 (1 cycle/row on the PE at any free size, vs fp32r's
2-4 and heavier power throttling) with fp32 PSUM accumulation; weights and
hidden states are cast to bf16 host-side, which also halves HBM traffic.
"""
import math
import sys
import types

import ml_dtypes
import numpy as np

# ---------------------------------------------------------------------------
# Container compat: this walrus build rejects instructions carrying more than
# one sync-wait command. Patch Tile to (a) split multi-wait instructions into
# single-wait NoOps on the same engine, (b) hoist the end-of-kernel drain's
# waits onto single-wait NOPs. Also register the NTFF profile hook (the
# image's antenv lacks axon_hooks) so trace=True works for profiling.
# ---------------------------------------------------------------------------
import concourse.bass as bass
import concourse.mybir as mybir
import concourse.tile as tile
from concourse.bass_utils import run_bass_kernel_spmd
from concourse.tile import ScopedClock
from bass_rust import VectorClock

N_PROCS = len(VectorClock())
_PATCHED = False


def _install_ntff_hook():
    if 'antenv.axon_hooks' in sys.modules:
        return
    m = types.ModuleType('antenv.axon_hooks')
    holder = [None]
    m.set_axon_ntff_profile_hook = lambda h: holder.__setitem__(0, h)
    m.get_axon_ntff_profile_hook = lambda: holder[0]
    sys.modules['antenv.axon_hooks'] = m
    try:
        from trn_agent_boot.trn_boot import _ntff_profile_via_ctypes
        m.set_axon_ntff_profile_hook(
            _ntff_profile_via_ctypes('/opt/axon/libaxon_pjrt.so'))
    except Exception:
        pass


def _patched_drain_and_barrier(self, tick_clock, wait_clock):
    gc = tick_clock.global_clock
    for p in range(N_PROCS):
        if gc[p] == 0:
            continue
        single = VectorClock([gc[q] if q == p else 0 for q in range(N_PROCS)])
        nop_inst = self.nc.sync.nop(nofuse=True)
        wait_clock.add_sem_waits(nop_inst.ins, ScopedClock({None: single}))
    self.nc.sync.drain()
    self.nc.all_engine_barrier()
    popped = self.nc._tile_sem_poison_stack.pop()
    assert popped is self._sem_poison
    self.nc.clear_and_free_semaphores(list(self.sems.allocated().values()))
    self.nc.all_engine_barrier()


def _make_split_lower(orig):
    def _split_multi_waits(self, ordered):
        nc = self.nc
        for bb_name, insts in ordered.items():
            out = []
            for inst in insts:
                si = inst.sync_info
                waits = list(si.on_wait) if si is not None else []
                if len(waits) > 1:
                    for w in waits[:-1]:
                        nop = mybir.InstNoOp(
                            name=f"{inst.name}-waitsplit-{nc.next_id()}",
                            engine=inst.engine,
                            sync_info=mybir.SyncInfo(on_wait=[w], on_update=[]),
                        )
                        nc.register_instruction(nop)
                        out.append(nop)
                    inst.sync_info = mybir.SyncInfo(
                        on_wait=[waits[-1]], on_update=list(si.on_update))
                out.append(inst)
            ordered[bb_name] = out
        return orig(self, ordered)
    return _split_multi_waits


def _install_patches():
    global _PATCHED
    _install_ntff_hook()
    if _PATCHED:
        return
    tile.TileContext._drain_and_barrier = _patched_drain_and_barrier
    tile.TileContext._lower_ordered_insts = _make_split_lower(
        tile.TileContext._lower_ordered_insts)
    _PATCHED = True


_install_patches()

# ---------------------------------------------------------------------------
# Problem constants (hardcoded per the spec).
# ---------------------------------------------------------------------------
S = 2048
D = 2048
H = 16
KV = 4
GROUPS = H // KV
DN = 128          # d_nope
DR = 64           # d_rope
DQK = DN + DR     # 192
DV = 128
QR = 1536         # q rank
KVR = 512         # kv rank
EPS = 1e-6
NC_ = 8           # cores
TPC = S // NC_    # 256 tokens per core
NB = S // 128     # 16 k-subtiles
SCALE = 1.0 / math.sqrt(DQK)
NEG = -1e30

F32 = mybir.dt.float32
F32R = mybir.dt.float32r
BF16 = mybir.dt.bfloat16

_BUILT = None     # cached (nc,) so repeat kernel() calls skip rebuild
LAST_RESULTS = None  # BassKernelResults stash for test.py


def _emit_sum_av(nc, ps_sum, ps_av, v_sb, hk, ones_b, kb, psl, p_t):
    nc.tensor.matmul(ps_sum[:, psl], ones_b[:], p_t[:, psl],
                     start=(kb == 0), stop=(kb == NB - 1))
    nc.tensor.matmul(ps_av[:, psl], v_sb[kb][:, hk * 128:(hk + 1) * 128],
                     p_t[:, psl], start=(kb == 0), stop=(kb == NB - 1))


def _build():
    nc = bass.Bass()

    # ---- DRAM I/O (identical declaration on all cores; data differs) ----
    hT = nc.dram_tensor("hT", [D, S], BF16, kind="ExternalInput")
    hTq = nc.dram_tensor("hTq", [D, TPC], BF16, kind="ExternalInput")
    wqa = nc.dram_tensor("wqa", [D, QR], BF16, kind="ExternalInput")
    wqbm = nc.dram_tensor("wqbm", [QR, H * DQK], BF16, kind="ExternalInput")
    wkva = nc.dram_tensor("wkva", [D, KVR + DR], BF16, kind="ExternalInput")
    wkvbk = nc.dram_tensor("wkvbk", [KVR, KV * DN], BF16, kind="ExternalInput")
    wkvbv = nc.dram_tensor("wkvbv", [KVR, KV * DV], BF16, kind="ExternalInput")
    wo_t = nc.dram_tensor("wo", [H * DV, D], BF16, kind="ExternalInput")
    cossinT = nc.dram_tensor("cossinT", [2 * DR, S], F32, kind="ExternalInput")
    cosq2 = nc.dram_tensor("cosq2", [2 * DR, TPC], BF16, kind="ExternalInput")
    sinq2 = nc.dram_tensor("sinq2", [2 * DR, TPC], BF16, kind="ExternalInput")
    bmask = nc.dram_tensor("bmask", [128, 16 * 64], F32, kind="ExternalInput")
    out = nc.dram_tensor("out", [TPC, D], F32, kind="ExternalOutput")
    # scratch for free->partition broadcasts
    scr_k = nc.dram_tensor("scr_k", [1, S], F32, kind="Internal")
    scr_q = nc.dram_tensor("scr_q", [1, TPC], F32, kind="Internal")
    scr_r = nc.dram_tensor("scr_r", [8, 2 * TPC], F32, kind="Internal")

    def bcast_src(dram, off, ncols):
        # element off.. of the flat DRAM vector, broadcast to 128 partitions
        ap = dram[:]
        return bass.AP(tensor=ap.tensor, offset=ap.offset + off,
                       ap=[[0, 128], [1, ncols]])

    def strided(ap_src, start, stride, count):
        # free-dim strided view of a full 2D sbuf/psum tile AP
        return bass.AP(tensor=ap_src.tensor, offset=ap_src.offset + start,
                       ap=[ap_src.ap[0], [stride, count]])

    with tile.TileContext(nc) as tc:
        with (
            tc.tile_pool(name="persist", bufs=1) as P,   # attention-lived
            tc.tile_pool(name="wstream", bufs=6) as WS,  # streamed weights
            tc.tile_pool(name="ppool", bufs=6) as PP,    # p tiles (bf16)
        ):
            ones_b = P.tile([128, 1], BF16, name="ones_b")
            nc.vector.memset(ones_b[:], 1.0)
            eps_sb = P.tile([1, 1], F32, name="eps_sb")
            nc.vector.memset(eps_sb[:], EPS)
            # bmask[k, kb, i]: causal band masks over the 64 interleaved
            # packed columns at the diagonal of key block kb
            bmask_sb = P.tile([128, 16, 64], F32, name="bmask_sb")
            nc.sync.dma_start(bmask_sb[:], bmask[:, :].rearrange(
                "k (r i) -> k r i", r=16))
            # per-key-partition exp scale: SCALE / rms_denom(key token), so
            # k_nope stays raw in SBUF and the k-side rms scale is applied
            # inside the exp activation (krot2 is pre-divided to compensate)
            expsc = P.tile([128, 16], F32, name="expsc")

            # attention-lived products; qnT2[j] holds the pair (2j, 2j+1)
            # nope queries interleaved: col 2q+p <-> (query q, head 2j+p)
            qnT2 = [P.tile([128, 2 * TPC], BF16, name=f"qnT2{j}")
                    for j in range(8)]
            # qr2[j]: roped queries, zero-padded interleave — rows 0:64 hold
            # head 2j at even cols, rows 64:128 head 2j+1 at odd cols, so one
            # matmul against the duplicated krot2 yields both heads' rope
            # scores at the packed columns.
            qr2 = [P.tile([128, 2 * TPC], BF16, name=f"qr2{j}")
                   for j in range(8)]
            for j in range(8):
                nc.vector.memset(qr2[j][:], 0.0)
            knopeT = [P.tile([128, S], BF16, name=f"knopeT{h}")
                      for h in range(KV)]
            v_sb = [P.tile([128, KV * DV], BF16, name=f"v{m}")
                    for m in range(16)]
            # k_rot^T duplicated in both partition halves so the rope scores
            # matmul can match base_partition with either half of a q pair
            krot2 = P.tile([128, S], BF16, name="krot2")

            # ========================= Q window =========================
            with (
                tc.tile_pool(name="qwin", bufs=1) as QW,
                tc.tile_pool(name="qsc", bufs=2) as QS,
            ):
                # q_a^T [1536, 256] bf16 (raw, pre-rms)
                qaT = [QW.tile([128, TPC], BF16, name=f"qaT{m}")
                       for m in range(12)]
                with tc.tile_pool(name="qaps", bufs=1, space="PSUM") as PSB:
                    for half in range(2):
                        pss = [PSB.tile([128, TPC], F32, name=f"ps_qa{m}",
                                        tag=f"ps_qa{m}") for m in range(6)]
                        for k in range(16):
                            wch = WS.tile([128, 768], BF16, name="wch",
                                          tag="wch")
                            nc.gpsimd.dma_start(
                                wch[:], wqa[k * 128:(k + 1) * 128,
                                            half * 768:(half + 1) * 768])
                            hch = QS.tile([128, TPC], BF16, name="hqch",
                                          tag="hqch", bufs=6)
                            nc.sync.dma_start(
                                hch[:], hTq[k * 128:(k + 1) * 128, :])
                            for m in range(6):
                                nc.tensor.matmul(
                                    pss[m][:], wch[:, m * 128:(m + 1) * 128],
                                    hch[:], start=(k == 0), stop=(k == 15))
                        for m in range(6):
                            nc.vector.tensor_copy(qaT[half * 6 + m][:],
                                                  pss[m][:])

                    # q RMS scale vector (applied at q_b evac: per-token
                    # scaling commutes through the matmul)
                    ps_qss = PSB.tile([1, TPC], F32, name="ps_qss")
                    for m in range(12):
                        sq = QS.tile([128, TPC], BF16, name="sqq", tag="sqq")
                        nc.scalar.activation(
                            sq[:], qaT[m][:],
                            mybir.ActivationFunctionType.Square)
                        nc.tensor.matmul(ps_qss[:], ones_b[:], sq[:],
                                         start=(m == 0), stop=(m == 11))
                    srt_q = QW.tile([1, TPC], F32, name="srt_q")
                    nc.scalar.activation(srt_q[:], ps_qss[:],
                                         mybir.ActivationFunctionType.Sqrt,
                                         bias=eps_sb[:], scale=1.0 / QR)
                    nc.sync.dma_start(scr_q[:], srt_q[:])
                    qsb = QW.tile([128, TPC], F32, name="qsb")
                    nc.sync.dma_start(qsb[:], bcast_src(scr_q, 0, TPC))
                    qscale_bc = QW.tile([128, TPC], F32, name="qscale_bc")
                    nc.vector.reciprocal(qscale_bc[:], qsb[:])

                # q_b: nope per head + rope pairs; rms scale applied at evac
                qrT = [QW.tile([128, TPC], BF16, name=f"qrT{j}")
                       for j in range(8)]
                with tc.tile_pool(name="qbps", bufs=1, space="PSUM") as PSB:
                    for g in range(4):
                        psn = [PSB.tile([128, TPC], F32, name=f"ps_qb{u}",
                                        tag=f"ps_qb{u}") for u in range(6)]
                        for k in range(12):
                            wch = WS.tile([128, 768], BF16, name="wch",
                                          tag="wch")
                            nc.gpsimd.dma_start(
                                wch[:],
                                wqbm[k * 128:(k + 1) * 128,
                                     g * 768:(g + 1) * 768])
                            for l in range(4):
                                nc.tensor.matmul(
                                    psn[l][:], wch[:, l * 128:(l + 1) * 128],
                                    qaT[k][:], start=(k == 0), stop=(k == 11))
                            for lj in range(2):
                                nc.tensor.matmul(
                                    psn[4 + lj][:],
                                    wch[:, 512 + lj * 128:512 + (lj + 1) * 128],
                                    qaT[k][:], start=(k == 0), stop=(k == 11))
                        for l in range(4):
                            nc.vector.tensor_mul(
                                strided(qnT2[2 * g + l // 2][:], l % 2, 2,
                                        TPC),
                                psn[l][:], qscale_bc[:])
                        for lj in range(2):
                            nc.vector.tensor_mul(qrT[g * 2 + lj][:],
                                                 psn[4 + lj][:],
                                                 qscale_bc[:])

                # RoPE on q pairs (rows 0-63 head 2j, 64-127 head 2j+1).
                # out = x*cos2 + rot(x)*sin2 with rot = partition rotate by
                # 32 within each 64-row block (via sbuf->sbuf DMA) and the
                # rotate_half sign folded into sin2 host-side.
                cosq_sb = QW.tile([128, TPC], BF16, name="cosq_sb")
                sinq_sb = QW.tile([128, TPC], BF16, name="sinq_sb")
                nc.sync.dma_start(cosq_sb[:], cosq2[:, :])
                nc.sync.dma_start(sinq_sb[:], sinq2[:, :])
                for j in range(8):
                    xr = QS.tile([128, TPC], BF16, name="xr", tag="xr")
                    for b0, b1 in ((0, 32), (32, 0), (64, 96), (96, 64)):
                        nc.scalar.dma_start(xr[b0:b0 + 32, :],
                                            qrT[j][b1:b1 + 32, :])
                    t1 = QS.tile([128, TPC], F32, name="t1q", tag="t1q")
                    nc.vector.tensor_mul(t1[:], qrT[j][:], cosq_sb[:])
                    nc.vector.tensor_mul(xr[:], xr[:], sinq_sb[:])
                    # interleave into qr2: head 2j -> rows 0:64 even cols,
                    # head 2j+1 -> rows 64:128 odd cols
                    nc.vector.tensor_add(
                        strided(qr2[j][0:64, :], 0, 2, TPC),
                        t1[0:64, :], xr[0:64, :])
                    nc.vector.tensor_add(
                        strided(qr2[j][64:128, :], 1, 2, TPC),
                        t1[64:128, :], xr[64:128, :])

            # ========================= KV window =========================
            # fully chunked over 4 token chunks of 512: a-proj -> rms ->
            # rope -> k_nope^T -> v, per chunk.
            with (
                tc.tile_pool(name="kvwin", bufs=1) as KW,
                tc.tile_pool(name="kvch", bufs=2) as KC,
                tc.tile_pool(name="ksc", bufs=2) as KS,
                tc.tile_pool(name="kps", bufs=1, space="PSUM") as PSB,
            ):
                wkva_sb = [KW.tile([128, KVR + DR], BF16, name=f"wkva{k}")
                           for k in range(16)]
                for k in range(16):
                    nc.gpsimd.dma_start(wkva_sb[k][:],
                                        wkva[k * 128:(k + 1) * 128, :])
                wkvbk_sb = [KW.tile([128, KV * DN], BF16, name=f"wkvbk{k}")
                            for k in range(4)]
                wkvbv_sb = [KW.tile([128, KV * DV], BF16, name=f"wkvbv{k}")
                            for k in range(4)]
                for k in range(4):
                    nc.gpsimd.dma_start(wkvbk_sb[k][:],
                                        wkvbk[k * 128:(k + 1) * 128, :])
                    nc.gpsimd.dma_start(wkvbv_sb[k][:],
                                        wkvbv[k * 128:(k + 1) * 128, :])

                m_sizes = [128, 128, 128, 128, 64]
                for n in range(4):
                    ncols = slice(n * 512, (n + 1) * 512)
                    # ---- a-projection for this chunk ----
                    ckv = [KC.tile([m_sizes[m], 512], BF16 if m < 4 else F32R,
                                   name=f"ckv{m}", tag=f"ckv{m}")
                           for m in range(5)]
                    pss = [PSB.tile([m_sizes[m], 512], F32, name=f"ps_kva{m}",
                                    tag=f"ps_kva{m}") for m in range(5)]
                    for k in range(16):
                        hch = WS.tile([128, 768], BF16, name="wch", tag="wch")
                        nc.gpsimd.dma_start(hch[:, 0:512],
                                            hT[k * 128:(k + 1) * 128, ncols])
                        for m in range(5):
                            nc.tensor.matmul(
                                pss[m][:],
                                wkva_sb[k][:, m * 128: m * 128 + m_sizes[m]],
                                hch[:, 0:512], start=(k == 0), stop=(k == 15))
                    for m in range(5):
                        nc.vector.tensor_copy(ckv[m][:], pss[m][:])

                    # ---- RoPE on k_rot (raw; no rms on the rope part):
                    # out = x*cos + rot(x)*sin_signed, rot via DMA ----
                    cos_t = KS.tile([64, 512], F32, name="cos_t", tag="cos_t",
                                    bufs=2)
                    sin_t = KS.tile([64, 512], F32, name="sin_t", tag="sin_t",
                                    bufs=2)
                    nc.sync.dma_start(cos_t[:], cossinT[0:64, ncols])
                    nc.sync.dma_start(sin_t[:], cossinT[64:128, ncols])
                    kxr = KS.tile([64, 512], F32R, name="kxr", tag="kxr")
                    nc.scalar.dma_start(kxr[0:32, :], ckv[4][32:64, :])
                    nc.scalar.dma_start(kxr[32:64, :], ckv[4][0:32, :])
                    kt1 = KS.tile([64, 512], F32, name="kt1", tag="kt1")
                    nc.vector.tensor_mul(kt1[:], ckv[4][:], cos_t[:])
                    nc.vector.tensor_mul(kxr[:], kxr[:], sin_t[:])
                    nc.vector.tensor_add(kt1[:], kt1[:], kxr[:])

                    # ---- RMS scale vector for this chunk ----
                    ps_ss = PSB.tile([1, 512], F32, name="ps_ssk",
                                     tag="ps_ssk")
                    for m in range(4):
                        sq = KS.tile([128, 512], BF16, name="sqk", tag="sqk")
                        nc.scalar.activation(
                            sq[:], ckv[m][:],
                            mybir.ActivationFunctionType.Square)
                        nc.tensor.matmul(ps_ss[:], ones_b[:], sq[:],
                                         start=(m == 0), stop=(m == 3))
                    srt_k = KS.tile([1, 512], F32, name="srt_k", tag="srt_k")
                    nc.scalar.activation(srt_k[:], ps_ss[:],
                                         mybir.ActivationFunctionType.Sqrt,
                                         bias=eps_sb[:], scale=1.0 / KVR)
                    nc.sync.dma_start(scr_k[:, ncols], srt_k[:])
                    ksb = KS.tile([128, 512], F32, name="ksb", tag="ksb")
                    nc.sync.dma_start(ksb[:], bcast_src(scr_k, n * 512, 512))
                    # krot2 pre-divided by the rms scale (x srt); the exp's
                    # per-partition scale multiplies it back
                    nc.vector.tensor_mul(krot2[0:64, ncols], kt1[:],
                                         ksb[0:64, :])
                    nc.scalar.dma_start(krot2[64:128, ncols],
                                        krot2[0:64, ncols])
                    # token-partition-shaped scale for v evac:
                    # [p, m] <-> token 128m + p of this chunk
                    kscaleT = KS.tile([128, 4], F32, name="kscaleT",
                                      tag="kscaleT")
                    skap = scr_k[:]
                    nc.sync.dma_start(
                        kscaleT[:],
                        bass.AP(tensor=skap.tensor,
                                offset=skap.offset + n * 512,
                                ap=[[1, 128], [128, 4]]))
                    nc.vector.reciprocal(kscaleT[:], kscaleT[:])
                    nc.scalar.activation(expsc[:, n * 4:(n + 1) * 4],
                                         kscaleT[:],
                                         mybir.ActivationFunctionType.Copy,
                                         scale=SCALE)

                    # ---- k_nope^T for this chunk (raw; scaled at exp) ----
                    for h in range(KV):
                        ps = PSB.tile([128, 512], F32, name="ps_kn",
                                      tag="ps_kn")
                        for k in range(4):
                            nc.tensor.matmul(
                                ps[:], wkvbk_sb[k][:, h * 128:(h + 1) * 128],
                                ckv[k][:], start=(k == 0), stop=(k == 3))
                        nc.vector.tensor_copy(knopeT[h][:, ncols], ps[:])

                    # ---- v natural for this chunk's 4 token tiles ----
                    for mm in range(4):
                        ps = PSB.tile([128, 512], F32, name="ps_v", tag="ps_v")
                        for k in range(4):
                            nc.tensor.matmul(
                                ps[:], ckv[k][:, mm * 128:(mm + 1) * 128],
                                wkvbv_sb[k][:], start=(k == 0), stop=(k == 3))
                        nc.vector.tensor_scalar_mul(v_sb[n * 4 + mm][:],
                                                    ps[:],
                                                    kscaleT[:, mm:mm + 1])

            # =========================== Attention ==========================
            # Head-pair packing: pair j = heads (2j, 2j+1); packed column
            # 2q + p <-> (query q, head 2j+p). One nope-score / exp / sum /
            # AV instruction covers both heads (N up to 512); rope scores
            # stay per-head (stride-2 PSUM dst). Causal slicing at 32-query
            # granularity: key block kb only needs queries >= 32*(kb//2).
            attn_T = [P.tile([128, TPC], BF16, name=f"attnT{h}")
                      for h in range(H)]

            with (
                tc.tile_pool(name="aps", bufs=2, space="PSUM") as PSA,
                tc.tile_pool(name="recb", bufs=4) as RB,
            ):
                for j in range(8):
                    hk = j // 2          # kv head for this pair
                    ps_av = PSA.tile([128, 2 * TPC], F32, name="ps_av",
                                     tag="ps_av")
                    ps_sum = PSA.tile([1, 2 * TPC], F32, name="ps_sum",
                                      tag="ps_sum")
                    # software-pipelined: emit kb's scores/exp, then the
                    # PREVIOUS kb's sum/AV, so the PE streams scores while
                    # the scalar engine runs exp
                    pend = []
                    for kb in range(NB):
                        c0 = 32 * (kb // 2)
                        psl = slice(2 * c0, 2 * TPC)
                        kcols = slice(kb * 128, (kb + 1) * 128)
                        ps_sc = PSA.tile([128, 2 * TPC], F32, name="ps_sc",
                                         tag="ps_sc")
                        nc.tensor.matmul(ps_sc[:, psl], knopeT[hk][:, kcols],
                                         qnT2[j][:, psl], start=True,
                                         stop=False)
                        nc.tensor.matmul(ps_sc[:, psl], krot2[:, kcols],
                                         qr2[j][:, psl],
                                         start=False, stop=True)
                        nc.vector.tensor_add(ps_sc[:, 2 * c0:2 * c0 + 64],
                                             ps_sc[:, 2 * c0:2 * c0 + 64],
                                             bmask_sb[:, kb, :])
                        p_t = PP.tile([128, 2 * TPC], BF16, name="p_t",
                                      tag="p_t")
                        nc.scalar.activation(
                            p_t[:, psl], ps_sc[:, psl],
                            mybir.ActivationFunctionType.Exp,
                            scale=expsc[:, kb:kb + 1])
                        pend.append((kb, psl, p_t))
                        if len(pend) > 1:
                            _emit_sum_av(nc, ps_sum, ps_av, v_sb, hk, ones_b,
                                         *pend.pop(0))
                    _emit_sum_av(nc, ps_sum, ps_av, v_sb, hk, ones_b,
                                 *pend.pop(0))
                    # broadcast raw sums (contiguous DMA), reciprocal on the
                    # broadcast tile (128 lanes), de-interleave on the DVE
                    sum_sb = PP.tile([1, 2 * TPC], F32, name="sum_sb",
                                     tag="sum_sb", bufs=3)
                    nc.vector.tensor_copy(sum_sb[:], ps_sum[:])
                    nc.sync.dma_start(scr_r[j:j + 1, :], sum_sb[:])
                    rbs = RB.tile([128, 2 * TPC], F32, name="rbs", tag="rbs")
                    nc.sync.dma_start(rbs[:],
                                      bcast_src(scr_r, j * 2 * TPC, 2 * TPC))
                    rbr = RB.tile([128, 2 * TPC], F32, name="rbr", tag="rbr")
                    nc.vector.reciprocal(rbr[:], rbs[:])
                    for p in range(2):
                        nc.vector.tensor_mul(attn_T[2 * j + p][:],
                                             strided(ps_av[:], p, 2, TPC),
                                             strided(rbr[:], p, 2, TPC))

            # ============================ o_proj ============================
            with (
                tc.tile_pool(name="ops", bufs=1, space="PSUM") as PSB,
                tc.tile_pool(name="wop", bufs=3) as WO,
            ):
                pso = [PSB.tile([128, 512], F32, name=f"ps_o{i}")
                       for i in range(8)]
                for h in range(H):
                    wos = WO.tile([128, 2048], BF16, name="wos", tag="wos")
                    nc.gpsimd.dma_start(wos[:], wo_t[h * 128:(h + 1) * 128, :])
                    for n in range(4):
                        for m in range(2):
                            nc.tensor.matmul(
                                pso[n * 2 + m][:],
                                attn_T[h][:, m * 128:(m + 1) * 128],
                                wos[:, n * 512:(n + 1) * 512],
                                start=(h == 0), stop=(h == H - 1))
                for i in range(8):
                    n, m = i // 2, i % 2
                    osb = PP.tile([128, 512], F32, name="osb", tag="osb",
                                  bufs=2)
                    nc.vector.tensor_copy(osb[:], pso[i][:])
                    nc.sync.dma_start(
                        out[m * 128:(m + 1) * 128, n * 512:(n + 1) * 512],
                        osb[:])

    return nc


def kernel(hidden_states, cos, sin, wq_a, q_a_ln_w, wq_b, wkv_a, kv_a_ln_w,
           wkv_b, wo, cache_position, _trace=False):
    global _BUILT, LAST_RESULTS
    hidden_states = np.asarray(hidden_states, dtype=np.float32)
    cos = np.asarray(cos, dtype=np.float32)
    sin = np.asarray(sin, dtype=np.float32)
    wq_a = np.asarray(wq_a, dtype=np.float32)
    q_a_ln_w = np.asarray(q_a_ln_w, dtype=np.float32)
    wq_b = np.asarray(wq_b, dtype=np.float32)
    wkv_a = np.asarray(wkv_a, dtype=np.float32)
    kv_a_ln_w = np.asarray(kv_a_ln_w, dtype=np.float32)
    wkv_b = np.asarray(wkv_b, dtype=np.float32)
    wo = np.asarray(wo, dtype=np.float32)
    cp = np.asarray(cache_position).astype(np.int64)

    # ---- host-side prep (layout/sharding only) ----
    bf16 = ml_dtypes.bfloat16
    h = hidden_states[0]                       # [S, D]
    hT = np.ascontiguousarray(h.T).astype(bf16)  # [D, S]
    cos_sel = cos[0][cp]                       # [S, DR]
    sin_sel = sin[0][cp]
    cosT = np.ascontiguousarray(cos_sel.T)     # [DR, S]
    sinT = np.ascontiguousarray(sin_sel.T)
    # fold the rmsnorm elementwise weights into the b-projections
    wqb_eff = wq_b * q_a_ln_w[:, None]
    wqb_r3 = wqb_eff.reshape(QR, H, DQK)
    wqbn = wqb_r3[:, :, :DN].reshape(QR, H * DN)
    wqbr = wqb_r3[:, :, DN:].reshape(QR, H * DR)
    # merged per-group layout: [512 nope | 256 rope] x 4 groups
    wqbm = np.empty((QR, H * DQK), np.float32)
    for g in range(4):
        wqbm[:, g * 768:g * 768 + 512] = wqbn[:, g * 512:(g + 1) * 512]
        wqbm[:, g * 768 + 512:(g + 1) * 768] = wqbr[:, g * 256:(g + 1) * 256]
    wqbm = wqbm.astype(bf16)
    wkvb_eff = wkv_b * kv_a_ln_w[:, None]      # [KVR, KV*(DN+DV)]
    wkvb_r = wkvb_eff.reshape(KVR, KV, DN + DV)
    wkvbk = np.ascontiguousarray(wkvb_r[:, :, :DN].reshape(KVR, KV * DN)).astype(bf16)
    wkvbv = np.ascontiguousarray(wkvb_r[:, :, DN:].reshape(KVR, KV * DV)).astype(bf16)
    wo_c = np.ascontiguousarray(wo).astype(bf16)
    wqa_c = wq_a.astype(bf16)
    wkva_c = wkv_a.astype(bf16)

    sgn = np.concatenate([-np.ones(DR // 2), np.ones(DR // 2)]
                         ).astype(np.float32)[:, None]
    cossinT = np.ascontiguousarray(
        np.concatenate([cosT, sinT * sgn], axis=0))
    in_maps = []
    for c in range(NC_):
        toks = np.arange(c, S, NC_)            # this core's 256 tokens
        hTq = np.ascontiguousarray(hT[:, toks])
        cq = cosT[:, toks]
        sq = (sinT * sgn)[:, toks]
        cosq2 = np.ascontiguousarray(
            np.concatenate([cq, cq], axis=0)).astype(bf16)
        sinq2 = np.ascontiguousarray(
            np.concatenate([sq, sq], axis=0)).astype(bf16)
        # band masks over the diagonal 64 interleaved packed columns of key
        # block kb: column 2*(q - c0) + p <-> query q, c0 = 32*(kb//2);
        # bm[kb][k, i] = 0 if key 128kb+k <= token 8q+c else NEG
        r_ = np.arange(16)[:, None, None]
        k_ = np.arange(128)[None, :, None]
        i_ = np.arange(64)[None, None, :]
        q_ = 32 * (r_ // 2) + i_ // 2
        bm = np.where(128 * r_ + k_ <= 8 * q_ + c, 0.0, NEG).astype(np.float32)
        bm_dev = np.ascontiguousarray(
            bm.transpose(1, 0, 2).reshape(128, 16 * 64))
        in_maps.append({
            "hT": hT, "hTq": hTq, "wqa": wqa_c, "wqbm": wqbm,
            "wkva": wkva_c, "wkvbk": wkvbk, "wkvbv": wkvbv, "wo": wo_c,
            "cossinT": cossinT, "cosq2": cosq2, "sinq2": sinq2,
            "bmask": bm_dev,
        })

    if _BUILT is None:
        _BUILT = _build()
    nc = _BUILT

    res = run_bass_kernel_spmd(nc, in_maps, core_ids=list(range(NC_)),
                               trace=_trace)
    LAST_RESULTS = res

    out_full = np.empty((S, D), dtype=np.float32)
    for c in range(NC_):
        out_full[c::NC_] = res.results[c]["out"]   # row m <-> token 8m+c
    return out_full[None]                      # [1, S, D]

